# revision 27
# baseline (speedup 1.0000x reference)
"""Distributed causal multi-head attention for Trainium2 (8 NeuronCores).

Problem (hardcoded): x[2, 2048, 1024], 16 heads, head_dim 64, causal
softmax(QK^T/8)V then out-proj with bias. f32 in/out.

Sharding: tensor parallel on heads across all 8 cores (2 heads per core),
both batches processed on every core (batch = inner loop). The ctx
exchange before the out-projection is an 8-core AllToAll per batch:
core c contributes ctx^T[128 rows = heads {2c,2c+1}, 2048 q] chunked
along q into 8 slices of 256; after the AllToAll each core holds the
full 1024-row ctx^T for ITS 256-token q-slice and computes
out[q_slice, :] = ctx^T.T @ Wo + bo with the full Wo. An AllToAll
moves 1/4 the bytes of the AllGather pair it replaces (the collective
cost is dominated by output size), and only the second one (batch 1)
sits on the critical path.

Per-core, per-batch attention (identical numerics to the AllGather
version):
  - Q^T,K^T packed 2 heads x 64 dims into 128 partitions, V per head
  - scores transposed S^T[k,q] = K Q^T so the softmax denominator comes
    out of the PE via an appended ones-column on V
  - exp without max-subtraction (scores are O(2), safe in fp32/bf16)
  - causal mask applied post-exp as a 0/1 bf16 multiply (DVE 2x mode)
  - ctx^T accumulated per q-chunk, normalized with 1/den partition-
    broadcast via a 33-row selector matmul
All matmuls bf16 (fp32 PSUM accumulation).
"""

import numpy as np
import ml_dtypes

from concourse import bass, bacc, mybir
from concourse import tile
from concourse.bass_utils import run_bass_kernel_spmd

BF16 = mybir.dt.bfloat16
F32 = mybir.dt.float32
Act = mybir.ActivationFunctionType

B, S, D = 2, 2048, 1024
H, HD = 16, 64
NCORES = 8
HPC = H // NCORES    # 2 heads per core
CW = HPC * HD        # 128 columns per core
QS = S // NCORES     # 256: per-core q-slice for the out-proj
QC = 512             # q-chunk width in attention
KC = 128             # k-chunk width
NQ = S // QC         # 4
NKC = S // KC        # 16
KPQ = QC // KC       # 4 k-chunks per q-chunk
DCH = D // 128       # 8 contraction chunks of 128
OCH = D // 128       # 8 out-proj column blocks

_CACHE = {}


def _build_bass():
    nc = bacc.Bacc(
        "TRN2", target_bir_lowering=False, debug=False, num_devices=NCORES
    )
    # Tile under-syncs readers of async collective outputs (readback DMAs can
    # fire before the exchange lands); completion waits are attached post-Tile
    _ccs = []
    _rds = []
    _zeros = []
    _scats = []
    _cdeps = []   # (consumer_inst, [producer_insts]) to hard-order post-Tile

    # per-core external inputs (same shapes on every core: SPMD)
    xT0 = nc.declare_dram_parameter("xT0", [D, S], BF16, isOutput=False)
    xT1 = nc.declare_dram_parameter("xT1", [D, S], BF16, isOutput=False)
    wq = nc.declare_dram_parameter("wq", [D, CW], BF16, isOutput=False)
    wk = nc.declare_dram_parameter("wk", [D, CW], BF16, isOutput=False)
    wv = nc.declare_dram_parameter("wv", [D, CW], BF16, isOutput=False)
    wo = nc.declare_dram_parameter("wo", [D, D], BF16, isOutput=False)
    bo = nc.declare_dram_parameter("bo", [D, 1], F32, isOutput=False)
    msk = nc.declare_dram_parameter("msk", [128, KPQ, QC], BF16, isOutput=False)
    vones = nc.declare_dram_parameter("vones", [128, NKC, HPC, 1], BF16, isOutput=False)
    # selector for den broadcast: bc[m,q] = sum_k sel33[k,m]*den_pair[k,q]
    sel33 = nc.declare_dram_parameter("sel33", [33, 128], BF16, isOutput=False)
    # per-core scatter row indices for the sparse ReduceScatter exchange
    idx16 = nc.declare_dram_parameter("idx16", [128, 64], mybir.dt.int16, isOutput=False)
    # rows 0-1023 batch 0, rows 1024-2047 batch 1; columns = my q-slice
    outT = nc.declare_dram_parameter("outT", [B * D, QS], F32, isOutput=True)
    xT = [xT0, xT1]

    with tile.TileContext(nc) as tc:
        with tc.tile_pool(name="dram", bufs=1, space="DRAM") as dram:
            # Exchange: a sparse 8-core ReduceScatter per batch. cc_in
            # flat chunk j (rows [1024j, +1024)) is the full-d ctx for
            # q-slice j, with only this core's 128 rows (offset 128*core)
            # populated via dma_scatter_add; the rest are zeroed by DMA at
            # startup (explicit waits below order zeros -> scatter -> RS:
            # Tile under-syncs multi-writer comm inputs). RS(add) hands
            # core j the summed chunk j = full-depth ctx of its q-slice.
            # Reduce semantics make completion imply all peers' data landed
            # (an 8-core AllToAll exchanged the same bytes but raced).
            cc_in = [dram.tile([NCORES * DCH * 128, QS], BF16, name=f"cc_in{b}")
                     for b in range(B)]
            cc_out = [dram.tile([DCH * 128, QS], BF16, name=f"cc_out{b}")
                      for b in range(B)]

            with tc.tile_pool(name="persist", bufs=1) as pp:
                wq_sb = pp.tile([128, DCH, CW], BF16, tag="wq_sb")
                wk_sb = pp.tile([128, DCH, CW], BF16, tag="wk_sb")
                wv_sb = pp.tile([128, DCH, CW], BF16, tag="wv_sb")
                wo_sb = pp.tile([128, DCH, D], BF16, tag="wo_sb")
                bo_sb = pp.tile([128, OCH, 1], F32, tag="bo_sb")
                msk_sb = pp.tile([128, KPQ, QC], BF16, tag="msk_sb")
                sel_sb = pp.tile([33, 128], BF16, tag="sel_sb")
                idx_sb = pp.tile([128, 64], mybir.dt.int16, tag="idx_sb")
                zsrc = pp.tile([128, S], BF16, tag="zsrc")
                xT_sb = [pp.tile([128, DCH, S], BF16, tag=f"xT_sb{b}", name=f"xT_sb{b}")
                         for b in range(B)]
                F8 = mybir.dt.float8e4
                qTf8 = [pp.tile([128, S], F8, tag=f"qTf8{b}", name=f"qTf8{b}") for b in range(B)]
                kTf8 = [pp.tile([128, S], F8, tag=f"kTf8{b}", name=f"kTf8{b}") for b in range(B)]
                # DoubleRow operand layout: head h on partitions [32h,32h+32),
                # free dims (i, q) with contraction dim d = 32*i + (p - 32h)
                q8 = [pp.tile([64, 2, S], F8, tag=f"q8{b}", name=f"q8{b}") for b in range(B)]
                k8 = [pp.tile([64, 2, S], F8, tag=f"k8{b}", name=f"k8{b}") for b in range(B)]
                v_aug = [pp.tile([128, NKC, HPC, HD + 1], BF16, tag=f"v_aug{b}", name=f"v_aug{b}")
                         for b in range(B)]
                ctxu = [pp.tile([128, S], F32, tag=f"ctxu{b}", name=f"ctxu{b}") for b in range(B)]
                # den per batch: head 0 at partition 0, head 1 at partition
                # 32 (ACT writes must start at multiples of 32); rows 1-31
                # zeroed so the K=33 selector matmul can broadcast both heads
                # to output partitions 0-63 / 64-127 in one instruction
                den = [pp.tile([33, S], BF16, tag=f"den{b}", name=f"den{b}")
                       for b in range(B)]
                ctxT_sb = [pp.tile([128, DCH, QS], BF16, tag=f"ctxT_sb{b}", name=f"ctxT_sb{b}")
                           for b in range(B)]
                for b in range(B):
                    nc.vector.memset(den[b][:], 0.0)

                # DMA order matters for startup latency: the small
                # constants (mask, ones-column, selector, idxs) go FIRST --
                # the interleaved schedule reaches the first AV/mask ops at
                # ~18us, racing these if they queue behind the bulk loads --
                # then wq + x(b0) so the projections can start streaming,
                # wo/bo last
                _mskd = nc.sync.dma_start(msk_sb[:], msk[:])
                _vod = [nc.sync.dma_start(v_aug[b][:, :, :, HD:HD + 1], vones[:])
                        for b in range(B)]
                _seld = nc.sync.dma_start(sel_sb[:], sel33[:])
                nc.sync.dma_start(idx_sb[:], idx16[:])
                nc.sync.dma_start(wq_sb[:], wq.rearrange("(c p) w -> p c w", p=128))
                for c in range(DCH):
                    nc.sync.dma_start(xT_sb[0][:, c, :], xT0[c * 128:(c + 1) * 128, :])
                nc.sync.dma_start(wk_sb[:], wk.rearrange("(c p) w -> p c w", p=128))
                nc.sync.dma_start(wv_sb[:], wv.rearrange("(c p) w -> p c w", p=128))
                for c in range(DCH):
                    nc.sync.dma_start(xT_sb[1][:, c, :], xT1[c * 128:(c + 1) * 128, :])
                nc.sync.dma_start(wo_sb[:], wo.rearrange("(c p) w -> p c w", p=128))
                nc.sync.dma_start(bo_sb[:], bo.rearrange("(o p) z -> p o z", p=128))
                nc.vector.memset(zsrc[:], 0.0)
                for b in range(B):
                    for z in range(NCORES):
                        _zeros.append(nc.sync.dma_start(
                            cc_in[b][1024 * z:1024 * (z + 1), :]
                            .rearrange("(c p) q -> p c q", p=128),
                            zsrc.rearrange("p (c q) -> p c q", c=NCORES),
                        ))

                # All PSUM pools coexist (phases interleave): 2+4+2 banks.
                # mm_ps is shared by the projections and the out-proj (they
                # never contend: proj(b1) overlaps attn(b0), outproj(b0)
                # overlaps attn(b1)).
                with tc.tile_pool(name="mm_ps", bufs=2, space="PSUM") as mmp, \
                     tc.tile_pool(name="sc_ps", bufs=2, space="PSUM") as scp, \
                     tc.tile_pool(name="ctbc_ps", bufs=2, space="PSUM") as ctp, \
                     tc.tile_pool(name="es_pool", bufs=NKC // 2 + 2) as esp, \
                     tc.tile_pool(name="norm", bufs=2) as np_pool, \
                     tc.tile_pool(name="out_sb", bufs=3) as outs:

                    def proj_qk_j(b, w_sb, dst, dst8, j):
                        qs = slice(j * QC, (j + 1) * QC)
                        ps = mmp.tile([128, QC], F32, tag="mm")
                        for c in range(DCH):
                            nc.tensor.matmul(
                                ps[:],
                                w_sb[:, c, :],
                                xT_sb[b][:, c, qs],
                                start=(c == 0),
                                stop=(c == DCH - 1),
                            )
                        # x16 scaling keeps the fp8e4 mantissa in range; the
                        # exp scale divides the 256x out of the scores
                        cp = nc.vector.tensor_scalar_mul(dst[:, qs], ps[:], 16.0)
                        # one DMA per (head, half): SBUF free dims must not
                        # cross partitions, so each transfer is a plain
                        # partition-slice copy
                        for h in range(HPC):
                            for i in range(2):
                                r0 = 64 * h + 32 * i
                                rm = nc.sync.dma_start(
                                    dst8[32 * h:32 * h + 32, i, qs],
                                    dst[r0:r0 + 32, qs],
                                )
                                _cdeps.append((rm, [cp], f"rm{id(rm)}"))

                    def proj_v_t(b, t):
                        # V for this core's 2 heads, tokens [128t, 128t+128)
                        ps = mmp.tile([128, QC], F32, tag="mm")
                        for c in range(DCH):
                            nc.tensor.matmul(
                                ps[:, 0:128],
                                xT_sb[b][:, c, t * 128:(t + 1) * 128],
                                wv_sb[:, c, :],
                                start=(c == 0),
                                stop=(c == DCH - 1),
                            )
                        nc.vector.tensor_copy(
                            v_aug[b][:, t, :, 0:HD],
                            ps[:, 0:128].rearrange("p (h w) -> p h w", h=HPC),
                        )

                    def proj_piece(b, j):
                        # Q, K for q-chunk j plus the matching 4 V token-chunks
                        proj_qk_j(b, wq_sb, qTf8[b], q8[b], j)
                        proj_qk_j(b, wk_sb, kTf8[b], k8[b], j)
                        for t in range(4 * j, 4 * j + 4):
                            proj_v_t(b, t)

                    def attn_unit(b, h, j):
                        hp = slice(32 * h, 32 * h + 32)
                        nkc = (j + 1) * KPQ
                        qs = slice(j * QC, (j + 1) * QC)
                        es_tiles = []
                        for c0 in range(0, nkc, 2):
                            # two k-chunks share one 2-bank PSUM tile
                            # -> one exp instruction
                            st = scp.tile([128, 2, QC], F32, tag="st")
                            for i in range(2):
                                c = c0 + i
                                nc.tensor.matmul(
                                    st[:, i, :],
                                    k8[b][hp, :, c * KC:(c + 1) * KC],
                                    q8[b][hp, :, qs],
                                    start=True, stop=True,
                                    perf_mode=mybir.MatmulPerfMode.DoubleRow,
                                )
                            es = esp.tile([128, 2, QC], BF16, tag="es")
                            nc.scalar.activation(es[:], st[:], Act.Exp, scale=0.125 / 256.0)
                            if c0 >= j * KPQ:
                                r = c0 - j * KPQ
                                mm = nc.vector.tensor_mul(
                                    es[:], es[:], msk_sb[:, r:r + 2, :]
                                )
                                if not _cdeps or _cdeps[0][0] is not mm:
                                    if not any(d[0] is mm for d in _cdeps):
                                        if len([d for d in _cdeps if d[2] == "msk"]) == 0:
                                            _cdeps.append((mm, [_mskd], "msk"))
                            es_tiles.append(es)
                        ct = ctp.tile([HD + 1, QC], F32, tag="ct")
                        for c in range(nkc):
                            av = nc.tensor.matmul(
                                ct[:],
                                v_aug[b][:, c, h, :],
                                es_tiles[c // 2][:, c % 2, :],
                                start=(c == 0),
                                stop=(c == nkc - 1),
                            )
                            if len([d for d in _cdeps if d[2] == f"vo{b}"]) == 0:
                                _cdeps.append((av, [_vod[b]], f"vo{b}"))
                        nc.vector.tensor_copy(
                            ctxu[b][h * HD:h * HD + HD, qs], ct[0:HD, :]
                        )
                        nc.vector.tensor_copy(
                            den[b][h * 32:h * 32 + 1, qs],
                            ct[HD:HD + 1, :],
                        )

                    def norm_cc(b):
                        ctxn = np_pool.tile([128, S], BF16, tag="ctxn")
                        for j in range(NQ):
                            qs = slice(j * QC, (j + 1) * QC)
                            bc = ctp.tile([128, QC], F32, tag="ct")
                            bcm = nc.tensor.matmul(
                                bc[:], sel_sb[:], den[b][:, qs],
                                start=True, stop=True,
                            )
                            if len([d for d in _cdeps if d[2] == f"sel{b}"]) == 0:
                                _cdeps.append((bcm, [_seld], f"sel{b}"))
                            rb = np_pool.tile([128, QC], F32, tag="rb")
                            nc.vector.reciprocal(rb[:], bc[:])
                            nc.vector.tensor_mul(
                                ctxn[:, qs], ctxu[b][:, qs], rb[:]
                            )
                        # scatter this core's 128 ctx rows into its
                        # stripe of each q-slice chunk of the sparse RS
                        # input (piece i = ctxn[i%128, 256*(i//128):...])
                        _scats.append(nc.gpsimd.dma_scatter_add(
                            cc_in[b][:],
                            ctxn.rearrange("p (t q) -> p t q", t=NCORES),
                            idx_sb[:],
                            num_idxs=NCORES * 128,
                            num_idxs_reg=NCORES * 128,
                            elem_size=QS,
                        ))
                        _ccs.append(nc.gpsimd.collective_compute(
                            "ReduceScatter",
                            mybir.AluOpType.add,
                            replica_groups=[list(range(NCORES))],
                            ins=[cc_in[b].opt()],
                            outs=[cc_out[b].opt()],
                        ))

                    def readback(b):
                        _rds.append((nc.sync.dma_start(
                            ctxT_sb[b][:, :, :],
                            cc_out[b].rearrange("(c p) q -> p c q", p=128),
                        ), b))

                    def out_proj(b):
                        # outT[oc, q_slice] = Wo[:, oc]^T ctxT + bo[oc].
                        # PSUM comes from the scores pool: the rotation's WAR
                        # chain keeps these matmuls from being scheduler-
                        # hoisted into the middle of attention (where their
                        # exchange-readback wait would stall the in-order PE
                        # queue).
                        for o in range(OCH):
                            ps_t = scp.tile([128, 2, QC], F32, tag="st", name="ps_t")
                            ps = ps_t[:, 0, :]
                            for c in range(DCH):
                                nc.tensor.matmul(
                                    ps[:, 0:QS],
                                    wo_sb[:, c, o * 128:(o + 1) * 128],
                                    ctxT_sb[b][:, c, :],
                                    start=(c == 0),
                                    stop=(c == DCH - 1),
                                )
                            ot = outs.tile([128, QS], F32, tag="ot")
                            nc.scalar.activation(
                                ot[:], ps[:, 0:QS], Act.Identity, bias=bo_sb[:, o, :]
                            )
                            nc.sync.dma_start(
                                outT[b * D + o * 128:b * D + (o + 1) * 128, :],
                                ot[:],
                            )

                    # Emission order IS per-engine execution order; attention
                    # is ACT(exp)-bound, so projection pieces are threaded
                    # between attention units to fill PE gaps, and the batch-1
                    # exchange is issued before batch-0's out-proj so only the
                    # final out-proj trails the last AllToAll.
                    # Emission order IS per-engine execution order;
                    # attention is ACT(exp)-bound, so projection pieces are
                    # threaded between attention units to fill PE gaps, and
                    # the batch-1 exchange is issued before batch-0's
                    # out-proj so only the final out-proj trails the last
                    # ReduceScatter.
                    proj_piece(0, 0)
                    attn_unit(0, 0, 0)
                    proj_piece(0, 1)
                    attn_unit(0, 1, 0)
                    attn_unit(0, 0, 1)
                    proj_piece(0, 2)
                    attn_unit(0, 1, 1)
                    attn_unit(0, 0, 2)
                    proj_piece(0, 3)
                    attn_unit(0, 1, 2)
                    attn_unit(0, 0, 3)
                    proj_piece(1, 0)
                    attn_unit(0, 1, 3)
                    norm_cc(0)
                    readback(0)
                    proj_piece(1, 1)
                    attn_unit(1, 0, 0)
                    proj_piece(1, 2)
                    attn_unit(1, 1, 0)
                    attn_unit(1, 0, 1)
                    proj_piece(1, 3)
                    attn_unit(1, 1, 1)
                    attn_unit(1, 0, 2)
                    attn_unit(1, 1, 2)
                    attn_unit(1, 0, 3)
                    attn_unit(1, 1, 3)
                    norm_cc(1)
                    readback(1)
                    out_proj(0)
                    out_proj(1)

    # Hard-order under-synced producer->consumer pairs (Tile misses some
    # DMA-write -> reader deps): compute each producer's completion value on
    # its (rolling, shared) DMA semaphore by a program-order scan, then
    # attach sem-ge waits to the consumer.
    insts = [i for blk in nc.m.functions[0].blocks for i in blk.instructions]
    cum = {}
    done_val = {}   # id(mybir inst) -> {semkey: value}
    prod_ids = {id(z.ins) for z in _zeros}
    for con, prods, _tag in _cdeps:
        prod_ids |= {id(p.ins) for p in prods}
    for inst in insts:
        si = inst.sync_info
        for u in (si.on_update if si else None) or []:
            key = (u.ant_name, u.id)
            cum[key] = cum.get(key, 0) + (u.update_value or 0)
            if id(inst) in prod_ids:
                done_val.setdefault(id(inst), {})[key] = cum[key]

    def _attach(consumer, producers):
        need = {}
        for p in producers:
            for key, v in done_val.get(id(p.ins), {}).items():
                need[key] = max(need.get(key, 0), v)
        assert need, "producer not found in program"
        for (ant, sid), v in need.items():
            consumer.wait_op(bass.SemaphoreHandle(ant, sid), v, "sem-ge", check=False)

    for sc in _scats:
        _attach(sc, _zeros)
    for con, prods, _tag in _cdeps:
        _attach(con, prods)

    # attach completion waits: readback DMAs for batch b must observe the
    # b-th collective's completion semaphore
    upd0 = _ccs[0].ins.sync_info.on_update[0]
    upd1 = _ccs[1].ins.sync_info.on_update[0]
    assert (upd0.ant_name, upd0.id) == (upd1.ant_name, upd1.id), (
        "collectives use distinct sems; adjust wait thresholds"
    )
    cc_done_sem = bass.SemaphoreHandle(upd0.ant_name, upd0.id)
    for rd, b in _rds:
        # check=False: wait slot may be taken; bacc splits into event sems
        rd.wait_op(cc_done_sem, b + 1, "sem-ge", check=False)
    nc.compile()
    return nc


def _causal_mask():
    # msk[kp, r, qf] = 1 where (r*128 + kp) <= qf else 0  (keep k <= q)
    kp = np.arange(128)[:, None, None]
    r = np.arange(KPQ)[None, :, None]
    qf = np.arange(QC)[None, None, :]
    return (r * 128 + kp <= qf).astype(ml_dtypes.bfloat16)


def _scatter_idxs(core):
    # piece i = (q-chunk t = i//128, d-row r = i%128); target flat row =
    # t*1024 + core*128 + r; idxs wrapped as idx[p, s] = idx for i=s*16+p%16
    idx = np.zeros((128, 64), dtype=np.int16)
    for i in range(NCORES * 128):
        t, r = i // 128, i % 128
        v = t * NCORES * 128 + core * 128 + r
        idx[i % 16, i // 16] = v
    idx[16:, :] = idx[:16, :].reshape(1, 16, 64).repeat(7, axis=0).reshape(112, 64)
    return idx


def _in_maps(x, Wq, Wk, Wv, Wo, bo):
    bf = ml_dtypes.bfloat16
    zbuf = np.zeros((NCORES * DCH * 128, QS), dtype=bf)
    msk = _causal_mask()
    sel33 = np.zeros((33, 128), dtype=bf)
    sel33[0, 0:64] = 1.0
    sel33[32, 64:128] = 1.0
    xT = [np.ascontiguousarray(x[b].T).astype(bf) for b in range(B)]
    wo_full = np.ascontiguousarray(Wo).astype(bf)
    bo_full = np.ascontiguousarray(bo[:, None]).astype(np.float32)
    vones = np.ones((128, NKC, HPC, 1), dtype=bf)
    maps = []
    for c in range(NCORES):
        cs = slice(c * CW, (c + 1) * CW)
        maps.append({
            "xT0": xT[0],
            "xT1": xT[1],
            "wq": np.ascontiguousarray(Wq[:, cs]).astype(bf),
            "wk": np.ascontiguousarray(Wk[:, cs]).astype(bf),
            "wv": np.ascontiguousarray(Wv[:, cs]).astype(bf),
            "wo": wo_full,
            "bo": bo_full,
            "msk": msk,
            "vones": vones,
            "sel33": sel33,
            "idx16": _scatter_idxs(c),
            "ccz0": zbuf,
            "ccz1": zbuf,
        })
    return maps


def kernel(x, Wq, Wk, Wv, Wo, bo, _trace=False):
    x = np.asarray(x, dtype=np.float32)
    Wq, Wk, Wv, Wo, bo = (np.asarray(a, dtype=np.float32) for a in (Wq, Wk, Wv, Wo, bo))
    if "nc" not in _CACHE:
        _CACHE["nc"] = _build_bass()
    nc = _CACHE["nc"]
    res = run_bass_kernel_spmd(
        nc, _in_maps(x, Wq, Wk, Wv, Wo, bo), list(range(NCORES)), trace=_trace
    )
    out = np.zeros((B, S, D), dtype=np.float32)
    for c in range(NCORES):
        qs = slice(c * QS, (c + 1) * QS)
        for b in range(B):
            out[b, qs, :] = res.results[c]["outT"][b * D:(b + 1) * D, :].T
    if _trace:
        return out, res
    return out


# revision 30
# speedup vs baseline: 1.0300x; 1.0300x over previous
"""Distributed causal multi-head attention for Trainium2 (8 NeuronCores).

Problem (hardcoded): x[2, 2048, 1024], 16 heads, head_dim 64, causal
softmax(QK^T/8)V then out-proj with bias. f32 in/out.

Sharding: tensor parallel on heads across all 8 cores (2 heads per core),
both batches processed on every core (batch = inner loop). The ctx
exchange before the out-projection is an 8-core AllToAll per batch:
core c contributes ctx^T[128 rows = heads {2c,2c+1}, 2048 q] chunked
along q into 8 slices of 256; after the AllToAll each core holds the
full 1024-row ctx^T for ITS 256-token q-slice and computes
out[q_slice, :] = ctx^T.T @ Wo + bo with the full Wo. An AllToAll
moves 1/4 the bytes of the AllGather pair it replaces (the collective
cost is dominated by output size), and only the second one (batch 1)
sits on the critical path.

Per-core, per-batch attention (identical numerics to the AllGather
version):
  - Q^T,K^T packed 2 heads x 64 dims into 128 partitions, V per head
  - scores transposed S^T[k,q] = K Q^T so the softmax denominator comes
    out of the PE via an appended ones-column on V
  - exp without max-subtraction (scores are O(2), safe in fp32/bf16)
  - causal mask applied post-exp as a 0/1 bf16 multiply (DVE 2x mode)
  - ctx^T accumulated per q-chunk, normalized with 1/den partition-
    broadcast via a 33-row selector matmul
All matmuls bf16 (fp32 PSUM accumulation).
"""

import numpy as np
import ml_dtypes

from concourse import bass, bacc, mybir
from concourse import tile
from concourse.bass_utils import run_bass_kernel_spmd

BF16 = mybir.dt.bfloat16
F32 = mybir.dt.float32
Act = mybir.ActivationFunctionType

B, S, D = 2, 2048, 1024
H, HD = 16, 64
NCORES = 8
HPC = H // NCORES    # 2 heads per core
CW = HPC * HD        # 128 columns per core
QS = S // NCORES     # 256: per-core q-slice for the out-proj
QC = 512             # q-chunk width in attention
KC = 128             # k-chunk width
NQ = S // QC         # 4
NKC = S // KC        # 16
KPQ = QC // KC       # 4 k-chunks per q-chunk
DCH = D // 128       # 8 contraction chunks of 128
OCH = D // 128       # 8 out-proj column blocks

_CACHE = {}


def _build_bass():
    nc = bacc.Bacc(
        "TRN2", target_bir_lowering=False, debug=False, num_devices=NCORES
    )
    # Tile under-syncs readers of async collective outputs (readback DMAs can
    # fire before the exchange lands); completion waits are attached post-Tile
    _ccs = []
    _rds = []
    _zeros = []
    _scats = []
    _cdeps = []   # (consumer_inst, [producer_insts]) to hard-order post-Tile

    # per-core external inputs (same shapes on every core: SPMD)
    xT0 = nc.declare_dram_parameter("xT0", [D, S], BF16, isOutput=False)
    xT1 = nc.declare_dram_parameter("xT1", [D, S], BF16, isOutput=False)
    wq = nc.declare_dram_parameter("wq", [D, CW], BF16, isOutput=False)
    wk = nc.declare_dram_parameter("wk", [D, CW], BF16, isOutput=False)
    wv = nc.declare_dram_parameter("wv", [D, CW], BF16, isOutput=False)
    wo = nc.declare_dram_parameter("wo", [D, D], BF16, isOutput=False)
    bo = nc.declare_dram_parameter("bo", [D, 1], F32, isOutput=False)
    msk = nc.declare_dram_parameter("msk", [128, KPQ, QC], BF16, isOutput=False)
    vones = nc.declare_dram_parameter("vones", [128, NKC, HPC, 1], BF16, isOutput=False)
    # selector for den broadcast: bc[m,q] = sum_k sel33[k,m]*den_pair[k,q]
    sel33 = nc.declare_dram_parameter("sel33", [33, 128], BF16, isOutput=False)
    # per-core scatter row indices for the sparse ReduceScatter exchange
    idx16 = nc.declare_dram_parameter("idx16", [128, 64], mybir.dt.int16, isOutput=False)
    # rows 0-1023 batch 0, rows 1024-2047 batch 1; columns = my q-slice
    outT = nc.declare_dram_parameter("outT", [B * D, QS], F32, isOutput=True)
    xT = [xT0, xT1]

    with tile.TileContext(nc) as tc:
        with tc.tile_pool(name="dram", bufs=1, space="DRAM") as dram:
            # Exchange: a sparse 8-core ReduceScatter per batch. cc_in
            # flat chunk j (rows [1024j, +1024)) is the full-d ctx for
            # q-slice j, with only this core's 128 rows (offset 128*core)
            # populated via dma_scatter_add; the rest are zeroed by DMA at
            # startup (explicit waits below order zeros -> scatter -> RS:
            # Tile under-syncs multi-writer comm inputs). RS(add) hands
            # core j the summed chunk j = full-depth ctx of its q-slice.
            # Reduce semantics make completion imply all peers' data landed
            # (an 8-core AllToAll exchanged the same bytes but raced).
            cc_in = [dram.tile([NCORES * DCH * 128, QS], BF16, name=f"cc_in{b}")
                     for b in range(B)]
            cc_out = [dram.tile([DCH * 128, QS], BF16, name=f"cc_out{b}")
                      for b in range(B)]

            with tc.tile_pool(name="persist", bufs=1) as pp:
                wq_sb = pp.tile([128, DCH, CW], BF16, tag="wq_sb")
                wk_sb = pp.tile([128, DCH, CW], BF16, tag="wk_sb")
                wv_sb = pp.tile([128, DCH, CW], BF16, tag="wv_sb")
                wo_sb = pp.tile([128, DCH, D], BF16, tag="wo_sb")
                bo_sb = pp.tile([128, OCH, 1], F32, tag="bo_sb")
                msk_sb = pp.tile([128, KPQ, QC], BF16, tag="msk_sb")
                sel_sb = pp.tile([33, 128], BF16, tag="sel_sb")
                idx_sb = pp.tile([128, 64], mybir.dt.int16, tag="idx_sb")
                zsrc = pp.tile([128, S], BF16, tag="zsrc")
                xT_sb = [pp.tile([128, DCH, S], BF16, tag=f"xT_sb{b}", name=f"xT_sb{b}")
                         for b in range(B)]
                F8 = mybir.dt.float8e4
                qTf8 = [pp.tile([128, S], F8, tag=f"qTf8{b}", name=f"qTf8{b}") for b in range(B)]
                kTf8 = [pp.tile([128, S], F8, tag=f"kTf8{b}", name=f"kTf8{b}") for b in range(B)]
                # DoubleRow operand layout: head h on partitions [32h,32h+32),
                # free dims (i, q) with contraction dim d = 32*i + (p - 32h)
                q8 = [pp.tile([64, 2, S], F8, tag=f"q8{b}", name=f"q8{b}") for b in range(B)]
                k8 = [pp.tile([64, 2, S], F8, tag=f"k8{b}", name=f"k8{b}") for b in range(B)]
                v_aug = [pp.tile([128, NKC, HPC, HD + 1], BF16, tag=f"v_aug{b}", name=f"v_aug{b}")
                         for b in range(B)]
                ctxu = [pp.tile([128, S], F32, tag=f"ctxu{b}", name=f"ctxu{b}") for b in range(B)]
                # den per batch: head 0 at partition 0, head 1 at partition
                # 32 (ACT writes must start at multiples of 32); rows 1-31
                # zeroed so the K=33 selector matmul can broadcast both heads
                # to output partitions 0-63 / 64-127 in one instruction
                den = [pp.tile([33, S], BF16, tag=f"den{b}", name=f"den{b}")
                       for b in range(B)]
                ctxT_sb = [pp.tile([128, DCH, QS], BF16, tag=f"ctxT_sb{b}", name=f"ctxT_sb{b}")
                           for b in range(B)]
                for b in range(B):
                    nc.vector.memset(den[b][:], 0.0)

                # DMA order matters for startup latency: the small
                # constants (mask, ones-column, selector, idxs) go FIRST --
                # the interleaved schedule reaches the first AV/mask ops at
                # ~18us, racing these if they queue behind the bulk loads --
                # then wq + x(b0) so the projections can start streaming,
                # wo/bo last
                _mskd = nc.sync.dma_start(msk_sb[:], msk[:])
                _vod = [nc.sync.dma_start(v_aug[b][:, :, :, HD:HD + 1], vones[:])
                        for b in range(B)]
                _seld = nc.sync.dma_start(sel_sb[:], sel33[:])
                nc.sync.dma_start(idx_sb[:], idx16[:])
                nc.sync.dma_start(wq_sb[:], wq.rearrange("(c p) w -> p c w", p=128))
                for c in range(DCH):
                    nc.sync.dma_start(xT_sb[0][:, c, :], xT0[c * 128:(c + 1) * 128, :])
                nc.sync.dma_start(wk_sb[:], wk.rearrange("(c p) w -> p c w", p=128))
                nc.sync.dma_start(wv_sb[:], wv.rearrange("(c p) w -> p c w", p=128))
                for c in range(DCH):
                    nc.sync.dma_start(xT_sb[1][:, c, :], xT1[c * 128:(c + 1) * 128, :])
                nc.sync.dma_start(wo_sb[:], wo.rearrange("(c p) w -> p c w", p=128))
                nc.sync.dma_start(bo_sb[:], bo.rearrange("(o p) z -> p o z", p=128))
                nc.vector.memset(zsrc[:], 0.0)
                for b in range(B):
                    for z in range(NCORES):
                        _zeros.append(nc.sync.dma_start(
                            cc_in[b][1024 * z:1024 * (z + 1), :]
                            .rearrange("(c p) q -> p c q", p=128),
                            zsrc.rearrange("p (c q) -> p c q", c=NCORES),
                        ))

                # All PSUM pools coexist (phases interleave): 2+4+2 banks.
                # mm_ps is shared by the projections and the out-proj (they
                # never contend: proj(b1) overlaps attn(b0), outproj(b0)
                # overlaps attn(b1)).
                with tc.tile_pool(name="mm_ps", bufs=2, space="PSUM") as mmp, \
                     tc.tile_pool(name="sc_ps", bufs=2, space="PSUM") as scp, \
                     tc.tile_pool(name="ctbc_ps", bufs=2, space="PSUM") as ctp, \
                     tc.tile_pool(name="es_pool", bufs=NKC // 2 + 2) as esp, \
                     tc.tile_pool(name="norm", bufs=2) as np_pool, \
                     tc.tile_pool(name="out_sb", bufs=3) as outs:

                    def proj_qk_j(b, w_sb, dst, dst8, j):
                        qs = slice(j * QC, (j + 1) * QC)
                        ps = mmp.tile([128, QC], F32, tag="mm")
                        for c in range(DCH):
                            nc.tensor.matmul(
                                ps[:],
                                w_sb[:, c, :],
                                xT_sb[b][:, c, qs],
                                start=(c == 0),
                                stop=(c == DCH - 1),
                            )
                        # x16 scaling keeps the fp8e4 mantissa in range; the
                        # exp scale divides the 256x out of the scores
                        cp = nc.vector.tensor_scalar_mul(dst[:, qs], ps[:], 16.0)
                        # one DMA per (head, half): SBUF free dims must not
                        # cross partitions, so each transfer is a plain
                        # partition-slice copy
                        for h in range(HPC):
                            for i in range(2):
                                r0 = 64 * h + 32 * i
                                # issued from the DVE queue: keeps these off
                                # the SP bulk-load queue (whose backlog would
                                # delay the first scores by ~35us)
                                rm = nc.scalar.dma_start(
                                    dst8[32 * h:32 * h + 32, i, qs],
                                    dst[r0:r0 + 32, qs],
                                )
                                _cdeps.append((rm, [cp], f"rm{id(rm)}"))

                    def proj_v_t(b, t):
                        # V for this core's 2 heads, tokens [128t, 128t+128)
                        ps = mmp.tile([128, QC], F32, tag="mm")
                        for c in range(DCH):
                            nc.tensor.matmul(
                                ps[:, 0:128],
                                xT_sb[b][:, c, t * 128:(t + 1) * 128],
                                wv_sb[:, c, :],
                                start=(c == 0),
                                stop=(c == DCH - 1),
                            )
                        nc.vector.tensor_copy(
                            v_aug[b][:, t, :, 0:HD],
                            ps[:, 0:128].rearrange("p (h w) -> p h w", h=HPC),
                        )

                    def proj_piece(b, j):
                        # Q, K for q-chunk j plus the matching 4 V token-chunks
                        proj_qk_j(b, wq_sb, qTf8[b], q8[b], j)
                        proj_qk_j(b, wk_sb, kTf8[b], k8[b], j)
                        for t in range(4 * j, 4 * j + 4):
                            proj_v_t(b, t)

                    def attn_unit(b, h, j):
                        hp = slice(32 * h, 32 * h + 32)
                        nkc = (j + 1) * KPQ
                        qs = slice(j * QC, (j + 1) * QC)
                        es_tiles = []
                        for c0 in range(0, nkc, 2):
                            # two k-chunks share one 2-bank PSUM tile
                            # -> one exp instruction
                            st = scp.tile([128, 2, QC], F32, tag="st")
                            for i in range(2):
                                c = c0 + i
                                nc.tensor.matmul(
                                    st[:, i, :],
                                    k8[b][hp, :, c * KC:(c + 1) * KC],
                                    q8[b][hp, :, qs],
                                    start=True, stop=True,
                                    perf_mode=mybir.MatmulPerfMode.DoubleRow,
                                )
                            es = esp.tile([128, 2, QC], BF16, tag="es")
                            nc.scalar.activation(es[:], st[:], Act.Exp, scale=0.125 / 256.0)
                            if c0 >= j * KPQ:
                                r = c0 - j * KPQ
                                mm = nc.vector.tensor_mul(
                                    es[:], es[:], msk_sb[:, r:r + 2, :]
                                )
                                if not _cdeps or _cdeps[0][0] is not mm:
                                    if not any(d[0] is mm for d in _cdeps):
                                        if len([d for d in _cdeps if d[2] == "msk"]) == 0:
                                            _cdeps.append((mm, [_mskd], "msk"))
                            es_tiles.append(es)
                        ct = ctp.tile([HD + 1, QC], F32, tag="ct")
                        for c in range(nkc):
                            av = nc.tensor.matmul(
                                ct[:],
                                v_aug[b][:, c, h, :],
                                es_tiles[c // 2][:, c % 2, :],
                                start=(c == 0),
                                stop=(c == nkc - 1),
                            )
                            if len([d for d in _cdeps if d[2] == f"vo{b}"]) == 0:
                                _cdeps.append((av, [_vod[b]], f"vo{b}"))
                        nc.vector.tensor_copy(
                            ctxu[b][h * HD:h * HD + HD, qs], ct[0:HD, :]
                        )
                        nc.vector.tensor_copy(
                            den[b][h * 32:h * 32 + 1, qs],
                            ct[HD:HD + 1, :],
                        )

                    def norm_cc(b):
                        ctxn = np_pool.tile([128, S], BF16, tag="ctxn")
                        for j in range(NQ):
                            qs = slice(j * QC, (j + 1) * QC)
                            bc = ctp.tile([128, QC], F32, tag="ct")
                            bcm = nc.tensor.matmul(
                                bc[:], sel_sb[:], den[b][:, qs],
                                start=True, stop=True,
                            )
                            if len([d for d in _cdeps if d[2] == f"sel{b}"]) == 0:
                                _cdeps.append((bcm, [_seld], f"sel{b}"))
                            rb = np_pool.tile([128, QC], F32, tag="rb")
                            nc.vector.reciprocal(rb[:], bc[:])
                            nc.vector.tensor_mul(
                                ctxn[:, qs], ctxu[b][:, qs], rb[:]
                            )
                        # scatter this core's 128 ctx rows into its
                        # stripe of each q-slice chunk of the sparse RS
                        # input (piece i = ctxn[i%128, 256*(i//128):...])
                        _scats.append(nc.gpsimd.dma_scatter_add(
                            cc_in[b][:],
                            ctxn.rearrange("p (t q) -> p t q", t=NCORES),
                            idx_sb[:],
                            num_idxs=NCORES * 128,
                            num_idxs_reg=NCORES * 128,
                            elem_size=QS,
                        ))
                        _ccs.append(nc.gpsimd.collective_compute(
                            "ReduceScatter",
                            mybir.AluOpType.add,
                            replica_groups=[list(range(NCORES))],
                            ins=[cc_in[b].opt()],
                            outs=[cc_out[b].opt()],
                        ))

                    def readback(b):
                        _rds.append((nc.sync.dma_start(
                            ctxT_sb[b][:, :, :],
                            cc_out[b].rearrange("(c p) q -> p c q", p=128),
                        ), b))

                    def out_proj(b):
                        # outT[oc, q_slice] = Wo[:, oc]^T ctxT + bo[oc].
                        # PSUM comes from the scores pool: the rotation's WAR
                        # chain keeps these matmuls from being scheduler-
                        # hoisted into the middle of attention (where their
                        # exchange-readback wait would stall the in-order PE
                        # queue).
                        for o in range(OCH):
                            ps_t = scp.tile([128, 2, QC], F32, tag="st", name="ps_t")
                            ps = ps_t[:, 0, :]
                            for c in range(DCH):
                                nc.tensor.matmul(
                                    ps[:, 0:QS],
                                    wo_sb[:, c, o * 128:(o + 1) * 128],
                                    ctxT_sb[b][:, c, :],
                                    start=(c == 0),
                                    stop=(c == DCH - 1),
                                )
                            ot = outs.tile([128, QS], F32, tag="ot")
                            nc.scalar.activation(
                                ot[:], ps[:, 0:QS], Act.Identity, bias=bo_sb[:, o, :]
                            )
                            nc.sync.dma_start(
                                outT[b * D + o * 128:b * D + (o + 1) * 128, :],
                                ot[:],
                            )

                    # Emission order IS per-engine execution order; attention
                    # is ACT(exp)-bound, so projection pieces are threaded
                    # between attention units to fill PE gaps, and the batch-1
                    # exchange is issued before batch-0's out-proj so only the
                    # final out-proj trails the last AllToAll.
                    # Emission order IS per-engine execution order;
                    # attention is ACT(exp)-bound, so projection pieces are
                    # threaded between attention units to fill PE gaps, and
                    # the batch-1 exchange is issued before batch-0's
                    # out-proj so only the final out-proj trails the last
                    # ReduceScatter.
                    proj_piece(0, 0)
                    attn_unit(0, 0, 0)
                    proj_piece(0, 1)
                    attn_unit(0, 1, 0)
                    attn_unit(0, 0, 1)
                    proj_piece(0, 2)
                    attn_unit(0, 1, 1)
                    attn_unit(0, 0, 2)
                    proj_piece(0, 3)
                    attn_unit(0, 1, 2)
                    attn_unit(0, 0, 3)
                    proj_piece(1, 0)
                    attn_unit(0, 1, 3)
                    norm_cc(0)
                    readback(0)
                    proj_piece(1, 1)
                    attn_unit(1, 0, 0)
                    proj_piece(1, 2)
                    attn_unit(1, 1, 0)
                    attn_unit(1, 0, 1)
                    proj_piece(1, 3)
                    attn_unit(1, 1, 1)
                    attn_unit(1, 0, 2)
                    attn_unit(1, 1, 2)
                    attn_unit(1, 0, 3)
                    attn_unit(1, 1, 3)
                    norm_cc(1)
                    readback(1)
                    out_proj(0)
                    out_proj(1)

    # Hard-order under-synced producer->consumer pairs (Tile misses some
    # DMA-write -> reader deps): compute each producer's completion value on
    # its (rolling, shared) DMA semaphore by a program-order scan, then
    # attach sem-ge waits to the consumer.
    insts = [i for blk in nc.m.functions[0].blocks for i in blk.instructions]
    cum = {}
    done_val = {}   # id(mybir inst) -> {semkey: value}
    prod_ids = {id(z.ins) for z in _zeros}
    for con, prods, _tag in _cdeps:
        prod_ids |= {id(p.ins) for p in prods}
    for inst in insts:
        si = inst.sync_info
        for u in (si.on_update if si else None) or []:
            key = (u.ant_name, u.id)
            cum[key] = cum.get(key, 0) + (u.update_value or 0)
            if id(inst) in prod_ids:
                done_val.setdefault(id(inst), {})[key] = cum[key]

    def _attach(consumer, producers):
        need = {}
        for p in producers:
            for key, v in done_val.get(id(p.ins), {}).items():
                need[key] = max(need.get(key, 0), v)
        assert need, "producer not found in program"
        for (ant, sid), v in need.items():
            consumer.wait_op(bass.SemaphoreHandle(ant, sid), v, "sem-ge", check=False)

    for sc in _scats:
        _attach(sc, _zeros)
    for con, prods, _tag in _cdeps:
        _attach(con, prods)

    # attach completion waits: readback DMAs for batch b must observe the
    # b-th collective's completion semaphore
    upd0 = _ccs[0].ins.sync_info.on_update[0]
    upd1 = _ccs[1].ins.sync_info.on_update[0]
    assert (upd0.ant_name, upd0.id) == (upd1.ant_name, upd1.id), (
        "collectives use distinct sems; adjust wait thresholds"
    )
    cc_done_sem = bass.SemaphoreHandle(upd0.ant_name, upd0.id)
    for rd, b in _rds:
        # check=False: wait slot may be taken; bacc splits into event sems
        rd.wait_op(cc_done_sem, b + 1, "sem-ge", check=False)
    nc.compile()
    return nc


def _causal_mask():
    # msk[kp, r, qf] = 1 where (r*128 + kp) <= qf else 0  (keep k <= q)
    kp = np.arange(128)[:, None, None]
    r = np.arange(KPQ)[None, :, None]
    qf = np.arange(QC)[None, None, :]
    return (r * 128 + kp <= qf).astype(ml_dtypes.bfloat16)


def _scatter_idxs(core):
    # piece i = (q-chunk t = i//128, d-row r = i%128); target flat row =
    # t*1024 + core*128 + r; idxs wrapped as idx[p, s] = idx for i=s*16+p%16
    idx = np.zeros((128, 64), dtype=np.int16)
    for i in range(NCORES * 128):
        t, r = i // 128, i % 128
        v = t * NCORES * 128 + core * 128 + r
        idx[i % 16, i // 16] = v
    idx[16:, :] = idx[:16, :].reshape(1, 16, 64).repeat(7, axis=0).reshape(112, 64)
    return idx


def _in_maps(x, Wq, Wk, Wv, Wo, bo):
    bf = ml_dtypes.bfloat16
    zbuf = np.zeros((NCORES * DCH * 128, QS), dtype=bf)
    msk = _causal_mask()
    sel33 = np.zeros((33, 128), dtype=bf)
    sel33[0, 0:64] = 1.0
    sel33[32, 64:128] = 1.0
    xT = [np.ascontiguousarray(x[b].T).astype(bf) for b in range(B)]
    wo_full = np.ascontiguousarray(Wo).astype(bf)
    bo_full = np.ascontiguousarray(bo[:, None]).astype(np.float32)
    vones = np.ones((128, NKC, HPC, 1), dtype=bf)
    maps = []
    for c in range(NCORES):
        cs = slice(c * CW, (c + 1) * CW)
        maps.append({
            "xT0": xT[0],
            "xT1": xT[1],
            "wq": np.ascontiguousarray(Wq[:, cs]).astype(bf),
            "wk": np.ascontiguousarray(Wk[:, cs]).astype(bf),
            "wv": np.ascontiguousarray(Wv[:, cs]).astype(bf),
            "wo": wo_full,
            "bo": bo_full,
            "msk": msk,
            "vones": vones,
            "sel33": sel33,
            "idx16": _scatter_idxs(c),
            "ccz0": zbuf,
            "ccz1": zbuf,
        })
    return maps


def kernel(x, Wq, Wk, Wv, Wo, bo, _trace=False):
    x = np.asarray(x, dtype=np.float32)
    Wq, Wk, Wv, Wo, bo = (np.asarray(a, dtype=np.float32) for a in (Wq, Wk, Wv, Wo, bo))
    if "nc" not in _CACHE:
        _CACHE["nc"] = _build_bass()
    nc = _CACHE["nc"]
    res = run_bass_kernel_spmd(
        nc, _in_maps(x, Wq, Wk, Wv, Wo, bo), list(range(NCORES)), trace=_trace
    )
    out = np.zeros((B, S, D), dtype=np.float32)
    for c in range(NCORES):
        qs = slice(c * QS, (c + 1) * QS)
        for b in range(B):
            out[b, qs, :] = res.results[c]["outT"][b * D:(b + 1) * D, :].T
    if _trace:
        return out, res
    return out
